# revision 1
# baseline (speedup 1.0000x reference)
"""CRF log-partition kernel for Trainium2 (8 NeuronCores, data-parallel batch).

Algorithm: the reference forward scan
    alpha' = logsumexp(alpha[None,:] + trans, axis=prev) + emit
is linearized to probability space:
    p' = (M @ p) * E,   M = exp(trans), E = exp(emit) * 2^-7
(the 2^-7 cancels the mean per-step log-growth of ~4.85, so the state
stays in f32/bf16 range with no renormalization; all scale bookkeeping
is recovered on the host from state snapshots).

Each batch item's 2048-step sequence is split into P=64 segments of L=32
steps scanned in parallel (products of positive matrices converge to
rank-1, so each segment's output direction is independent of its init;
scales are fixed up by a PREFIX-tick prefix-correction pass seeded with
the previous segment's final state — direction mismatch after 1 tick is
~5e-3 per seam, x63 seams => ~1e-4 rel err, far under tolerance). Per
core: 32 batch x 64 segments = 2048 chains laid out as X[128, 1024]
bf16 — tag-block A (chains 0-1023) on partitions 0-47, block B on
64-111 (junk rows are killed by zero rows of the padded [112,128]
lhsT).

E = exp(emissions)*2^-7 is precomputed on the HOST in bf16, already
transposed to the on-chip layout [tick, tag-slot(112), chain(1024)], so
the device pipeline is just: DMA -> per-tick (PE matmul [112x128
stationary, 512-col stream], DVE multiply straight from PSUM). No
on-device exp, no PE transposes, no PSUM->SBUF staging copies. Tick 0
(x = init * E_0) is folded into eT tile 0 by patching pinit into its
seg-0 columns. Per tick the 1024 columns run as two independent
512-col matmul->multiply chains (one PSUM bank each, QBUFS ping-pong)
so DVE stays saturated back-to-back; the cost-model cadence is
DVE-bound at ~1.32us/tick. "p"-type groups (ACT copy + Pool multiply;
GPSIMD cannot read PSUM) are supported but lose to the 3-hop chain
latency through the in-order engine queues.

Host stitches per-(batch,segment) log-scales in float64 from three bf16
snapshots per core: pass-1 state at tick PREFIX, pass-1 final, pass-2
final.

mask does not affect the forward value (m*x + (1-m)*x == x) and is ignored.
"""

import math

import numpy as np

B, S, T = 256, 2048, 48
NEG = -10000.0
NCORES = 8
BC = B // NCORES          # batch per core = 32
P = 64                    # segments per batch item
L = S // P                # ticks per segment = 32
PREFIX = 1                # prefix-correction ticks
C2POW = -7                # constant rescale folded into E
COLS = BC * P // 2        # chains per block (columns of X) = 1024
ROWS = 112                # meaningful partition rows (blocks at 0-47 / 64-111)
XROWS = 128               # physical tile rows
TPG = 1                   # ticks per resident eT tile (one DMA each)
QBUFS = 2                 # PSUM q bufs per column-group
XBUFS = 8
# per-tick column-groups: (engine, col0, col1); "v"=DVE, "p"=Pool
# "v": DVE multiplies straight from PSUM; "p": ACT copies q to SBUF bf16
# (GPSIMD cannot read PSUM), then Pool multiplies all-SBUF.
MGROUPS = (("v", 0, 512), ("v", 512, 1024))
PSKEW = 2                 # p-group issue lag (ticks) to avoid PE head-of-line

_CACHE = {}


def _build(**cfg):
    g = globals()
    saved = {k: g[k] for k in cfg}
    g.update(cfg)
    try:
        return _build_inner()
    finally:
        g.update(saved)


def _build_inner():
    from contextlib import ExitStack

    import concourse.bacc as bacc
    import concourse.bass as bass
    import concourse.mybir as mybir
    import concourse.tile as tile

    f32 = mybir.dt.float32
    bf16 = mybir.dt.bfloat16

    nc = bacc.Bacc(None, target_bir_lowering=False)

    NTILE = L // TPG
    eet_d = nc.dram_tensor("eet", [L, ROWS, COLS], bf16, kind="ExternalInput")
    w_d = nc.dram_tensor("wlhs", [ROWS, XROWS], bf16, kind="ExternalInput")
    wshift_d = nc.dram_tensor("wshift", [ROWS, XROWS], bf16,
                              kind="ExternalInput")
    snap16_d = nc.dram_tensor("snap16", [ROWS, COLS], bf16, kind="ExternalOutput")
    snapf_d = nc.dram_tensor("snapf", [ROWS, COLS], bf16, kind="ExternalOutput")
    snap2_d = nc.dram_tensor("snap2", [ROWS, COLS], bf16, kind="ExternalOutput")

    with tile.TileContext(nc) as tc:
        with ExitStack() as ctx:
            consts = ctx.enter_context(tc.tile_pool(name="consts", bufs=1))
            epool = ctx.enter_context(tc.tile_pool(name="epool", bufs=NTILE))
            xpool = ctx.enter_context(tc.tile_pool(name="xpool", bufs=XBUFS))
            qpool = ctx.enter_context(
                tc.tile_pool(name="qpool", bufs=QBUFS, space=bass.MemorySpace.PSUM))
            qbpool = ctx.enter_context(tc.tile_pool(name="qbpool", bufs=QBUFS))

            w_sb = consts.tile([ROWS, XROWS], bf16, tag="w")

            NG = len(MGROUPS)
            GW = [c1 - c0 for (_, c0, c1) in MGROUPS]
            lag = [0 if kind == "v" else PSKEW for (kind, _, _) in MGROUPS]

            wshift_sb = consts.tile([ROWS, XROWS], bf16, tag="wshift")

            # the w load leads the scalar ring so tick 1 isn't gated by the
            # bulk eT stream; eT tile 0 leads the sync ring
            nc.scalar.dma_start(w_sb[:], w_d[:])
            etiles = []
            for i in range(NTILE):
                et = epool.tile([XROWS, TPG * COLS], bf16, tag="e",
                                name=f"e{i}")
                src = eet_d[i * TPG:(i + 1) * TPG].rearrange("t p c -> p t c")
                # tiles 0-2 lead the sync ring back-to-back (tick k consumes
                # tile k, so early arrival order is critical); later tiles
                # alternate rings for HWDGE throughput
                dma_eng = nc.scalar if i > 2 and i % 2 else nc.sync
                dma_eng.dma_start(et[0:ROWS, :], src)
                etiles.append(et)
            # wshift is only needed at the pass-2 boundary — load it behind
            # the eT stream so it never delays early tiles
            nc.scalar.dma_start(wshift_sb[:], wshift_d[:])

            # tick 0 (x = init ⊙ E_0) is pre-applied on the HOST (p_init is
            # folded into eT tile 0's seg-0 columns), so the scan starts at
            # tick 1 reading eT tile 0 as the state.
            xg = [etiles[0][:, c0:c1] for (_, c0, c1) in MGROUPS]
            if PREFIX == 1:
                # state after tick 0 IS eT tile 0
                nc.sync.dma_start(snap16_d[:], etiles[0][0:ROWS, 0:COLS])

            def tick_g(gi, x_in, k, matmul):
                kind, c0, c1 = MGROUPS[gi]
                base = (k % TPG) * COLS
                esl = etiles[k // TPG][:, base + c0:base + c1]
                x_out = xpool.tile([XROWS, GW[gi]], bf16, tag=f"x{gi}",
                                   name=f"x{gi}_{k}")
                eng = nc.vector if kind == "v" else nc.gpsimd
                if not matmul:
                    eng.tensor_mul(x_out[:], x_in[:], esl)
                    return x_out
                q = qpool.tile([XROWS, GW[gi]], f32, tag=f"q{gi}",
                               name=f"q{gi}")
                nc.tensor.matmul(q[:], w_sb[:], x_in[0:ROWS, :])
                if kind == "v":
                    eng.tensor_mul(x_out[:], q[:], esl)
                else:
                    qb = qbpool.tile([XROWS, GW[gi]], bf16,
                                     tag=f"qb{gi}", name=f"qb{gi}")
                    nc.scalar.activation(
                        qb[:], q[:], mybir.ActivationFunctionType.Copy)
                    eng.tensor_mul(x_out[:], qb[:], esl)
                return x_out

            # pass 1; p-groups lag PSKEW ticks in issue order so their
            # slower chains never block the PE queue head for the v-groups
            for r in range(L + PSKEW):
                for gi, (kind, c0, c1) in enumerate(MGROUPS):
                    k = r - lag[gi]
                    if not 1 <= k < L:
                        continue
                    xg[gi] = tick_g(gi, xg[gi], k, matmul=True)
                    if k + 1 == PREFIX:
                        nc.sync.dma_start(snap16_d[0:ROWS, c0:c1],
                                          xg[gi][0:ROWS, :])
                    if k + 1 == L:
                        nc.sync.dma_start(snapf_d[0:ROWS, c0:c1],
                                          xg[gi][0:ROWS, :])

            # pass 2: init = pass-1 finals shifted by one 32-col segment slot
            x2g = []
            for gi in range(NG):
                w = GW[gi]
                x2 = xpool.tile([XROWS, w], bf16, tag=f"x{gi}",
                                name=f"x2_{gi}")
                if gi == 0:
                    nc.gpsimd.memset(x2[:, 0:32], 1.0)     # seg-0 slot unused
                    # block-B first seg init <- block-A last seg: partition
                    # shift rows 0-47 -> 64-111 via a permutation matmul +
                    # PSUM copy (a SBUF->SBUF DMA here costs ~2.2us of fixed
                    # DGE latency on the pass-boundary critical path)
                    qs = qpool.tile([XROWS, 32], f32, tag="qs", bufs=1)
                    nc.tensor.matmul(qs[:], wshift_sb[:],
                                     xg[NG - 1][0:ROWS, GW[NG - 1] - 32:])
                    nc.vector.tensor_copy(x2[64:112, 0:32], qs[64:112, :])
                else:
                    nc.vector.tensor_copy(
                        x2[:, 0:32], xg[gi - 1][:, GW[gi - 1] - 32:])
                nc.vector.tensor_copy(x2[:, 32:w], xg[gi][:, 0:w - 32])
                x2g.append(x2)
            for r in range(PREFIX + PSKEW):
                for gi, (kind, c0, c1) in enumerate(MGROUPS):
                    k = r - lag[gi]
                    if not 0 <= k < PREFIX:
                        continue
                    x2g[gi] = tick_g(gi, x2g[gi], k, matmul=True)
                    if k + 1 == PREFIX:
                        nc.sync.dma_start(snap2_d[0:ROWS, c0:c1],
                                          x2g[gi][0:ROWS, :])

    nc.compile()
    return nc


def _host_consts(transitions):
    """W lhsT, p_init (analytic first log-step), stitch constants."""
    import ml_dtypes

    tr = transitions.astype(np.float64)
    M = np.exp(tr)                                   # M[next, prev]
    wl = np.zeros((ROWS, XROWS), np.float64)
    wl[0:48, 0:48] = M.T                             # lhsT[k, m] = M[m, k]
    wl[64:112, 64:112] = M.T

    # analytic first step: v[next] = logsumexp_prev(tr[next, :] + alpha0)
    alpha0 = np.full(T, NEG, np.float64)
    alpha0[0] = 0.0
    sc = tr + alpha0[None, :]
    mm = sc.max(axis=1, keepdims=True)
    v = np.log(np.exp(sc - mm).sum(axis=1)) + mm[:, 0]
    vmax = v.max()
    p_init = np.exp(v - vmax)                        # [T]

    # seam shift: block-A rows t -> block-B rows 64+t (pass-2 B seeding)
    ws = np.zeros((ROWS, XROWS), np.float64)
    ws[np.arange(48), 64 + np.arange(48)] = 1.0

    bf = ml_dtypes.bfloat16
    w_np = wl.astype(bf)
    wshift_np = ws.astype(bf)

    r = tr[-1, :]
    r_max = r.max()
    w_last = np.exp(r - r_max)                       # final-row weights [T]
    return w_np, wshift_np, p_init, vmax, r_max, w_last


def _host_et(em_core, p_init):
    """[BC, S, T] f32 emissions -> [L, 128, COLS] bf16 exp'd + transposed.

    Chain (seg, b) lives at column 32*(seg % (P/2)) + b of tag-block
    seg // (P/2) (block A partitions 0-47, block B 64-111); tick k uses
    E of step seg*L + k. Junk tag-slot rows are 1.0 (finite filler).
    Tick 0 of the scan (x = init * E_0) is pre-applied: seg-0's columns
    of E_0 are scaled by p_init, so the device starts at tick 1 with eT
    tile 0 as the state.
    """
    import ml_dtypes

    e = np.exp(em_core.astype(np.float32)) * (2.0 ** C2POW)
    # [b, seg, L, t] -> [blk, s, L, t, b] with seg = blk*(P//2) + s
    e = e.reshape(BC, 2, P // 2, L, T).transpose(1, 2, 3, 4, 0)
    # -> [blk, L, t, s*32 + b]
    e = e.transpose(0, 2, 3, 1, 4).reshape(2, L, T, COLS)
    out = np.ones((L, ROWS, COLS), np.float32)
    out[:, 0:48] = e[0]
    out[:, 64:112] = e[1]
    out[0, 0:48, 0:32] *= p_init[:, None].astype(np.float32)
    return out.astype(ml_dtypes.bfloat16)


def _stitch(snap16, snapf, snap2, vmax, r_max, w_last):
    """Per-core host stitch -> [BC] log partition (float64)."""
    def tags(a):  # [112, COLS] -> [T, P, BC] per-chain tag values
        a = np.asarray(a, np.float64)
        return np.concatenate([a[0:48, :], a[64:112, :]], axis=1) \
                 .reshape(T, P, BC)                   # chain = seg*BC + b

    s16 = np.log(np.maximum(tags(snap16).sum(axis=0), 1e-300))   # [P, BC]
    last = tags(snapf)
    sf = np.log(np.maximum(last.sum(axis=0), 1e-300))
    s2 = np.log(np.maximum(tags(snap2).sum(axis=0), 1e-300))

    Lfin = sf[P - 1, :] + (s2[1:, :] - s16[1:, :]).sum(axis=0)

    fin = last[:, -1, :]                              # [T, BC] final-seg state
    d = np.log(np.maximum((w_last[:, None] * fin).sum(axis=0), 1e-300)) \
        - np.log(np.maximum(fin.sum(axis=0), 1e-300))

    return Lfin + d + r_max + vmax - S * C2POW * math.log(2.0)


def _in_maps(emissions, transitions):
    w_np, wshift_np, p_init, vmax, r_max, w_last = _host_consts(transitions)
    in_maps = []
    for c in range(NCORES):
        in_maps.append({
            "eet": _host_et(emissions[c * BC:(c + 1) * BC], p_init),
            "wlhs": w_np,
            "wshift": wshift_np,
        })
    return in_maps, (vmax, r_max, w_last)


def kernel(**inputs):
    emissions = np.ascontiguousarray(inputs["emissions"], dtype=np.float32)
    transitions = np.asarray(inputs["transitions"], dtype=np.float32)

    if "nc" not in _CACHE:
        _CACHE["nc"] = _build()
    nc = _CACHE["nc"]

    in_maps, (vmax, r_max, w_last) = _in_maps(emissions, transitions)

    from concourse.bass_utils import run_bass_kernel_spmd
    res = run_bass_kernel_spmd(nc, in_maps, list(range(NCORES))).results

    out = np.empty(B, np.float32)
    for c in range(NCORES):
        r = res[c]
        out[c * BC:(c + 1) * BC] = _stitch(
            r["snap16"], r["snapf"], r["snap2"], vmax, r_max, w_last
        ).astype(np.float32)
    return out

